# revision 2
# baseline (speedup 1.0000x reference)
"""MultiHeadLatentAttention Trainium2 Bass kernel.

Sharding (8 cores): core c = (b, hg) with b = c // 2, hg = c % 2.
Each core: batch b, head-group hg (8 of 16 heads).
  Phase 1: QKV proj (bf16, host-transposed x), rmsnorm (DVE) + rope,
           q^T/k^T/v kept SBUF-resident in bf16.
  Phase 2: causal attention, j-outer/hp-inner; batched exp from multi-bank
           PSUM; row-packed QK^T (2 heads via tile_position); N-trimmed
           diagonal blocks; per-j pairwise AllGather of y^T (bf16);
           out-projection interleaved to hide collective latency.
Host concatenates per-core output halves.
"""

import numpy as np

import concourse.bass as bass
import concourse.mybir as mybir
import concourse.tile as tile
from concourse import bacc
from concourse.bass import ts
from concourse.masks import make_identity

F32 = mybir.dt.float32
F32R = mybir.dt.float32r
BF16 = mybir.dt.bfloat16
AF = mybir.ActivationFunctionType

N_HEAD = 16
N_EMBD = 2048
N_LATENT = 1024
HEAD_DIM = 64
ROPE_BASE = 10000.0
EPS = 1e-6
N_CORES = 8

HPC = N_HEAD // 2        # heads per core = 8
DW = HPC * HEAD_DIM      # local head width = 512
T = 2048
C = 2048
NT = T // 128            # 16 t-tiles
NCT = C // 128           # 16 c-tiles
NJ = 4                   # q chunks of 512
CH = C // 2              # out c-half = 1024


def build_nc(T_=T, C_=C, num_devices=N_CORES):
    nc = bacc.Bacc("TRN2", target_bir_lowering=False, debug=False,
                   num_devices=num_devices)

    xT_d = nc.dram_tensor("xT", [C, T], BF16, kind="ExternalInput").ap()
    wqT_d = nc.dram_tensor("wqT", [C, DW], BF16, kind="ExternalInput").ap()
    wkT_d = nc.dram_tensor("wkT", [C, DW], BF16, kind="ExternalInput").ap()
    wvT_d = nc.dram_tensor("wvT", [C, DW], BF16, kind="ExternalInput").ap()
    woT_d = nc.dram_tensor("woutT", [N_LATENT, CH], BF16,
                           kind="ExternalInput").ap()
    cos_d = nc.dram_tensor("cosf", [T, DW], F32, kind="ExternalInput").ap()
    sin_d = nc.dram_tensor("sinf", [T, DW], F32, kind="ExternalInput").ap()
    tri_d = nc.dram_tensor("tri", [128, 128], BF16, kind="ExternalInput").ap()
    out_d = nc.dram_tensor("out_half", [T, CH], F32, kind="ExternalOutput").ap()

    groups = [[i, i + 1] for i in range(0, num_devices, 2)]

    with tile.TileContext(nc) as tc:
        with (
            tc.tile_pool(name="const", bufs=1) as constp,
            tc.tile_pool(name="keep", bufs=1) as keep,
            tc.tile_pool(name="dram", bufs=1, space=bass.MemorySpace.DRAM) as dramp,
        ):
            identf = constp.tile([128, 128], F32, tag="identf")
            make_identity(nc, identf[:])
            identb = constp.tile([128, 128], BF16, tag="identb")
            nc.vector.tensor_copy(identb[:], identf[:])
            eps_sb = constp.tile([128, 1], F32, tag="eps")
            nc.vector.memset(eps_sb[:], EPS)
            ones_f = constp.tile([128, 64], F32, tag="ones_f")
            nc.vector.memset(ones_f[:], 1.0)
            onesr = constp.tile([128, 64], F32R, tag="onesr")
            nc.vector.tensor_copy(onesr[:], ones_f[:])
            tri = constp.tile([128, 128], BF16, tag="tri")
            nc.sync.dma_start(tri[:], tri_d)

            # persistent bf16 q^T, k^T (d-major) and v (t-major, 65-col trick)
            qtd = [keep.tile([128, T], BF16, tag=f"qtd{db}", name=f"qtd{db}")
                   for db in range(4)]
            ktd = [keep.tile([128, T], BF16, tag=f"ktd{db}", name=f"ktd{db}")
                   for db in range(4)]
            v65 = [keep.tile([128, HPC * 65], BF16, tag=f"v65_{si}",
                             name=f"v65_{si}") for si in range(NT)]
            for si in range(NT):
                vv = v65[si][:].rearrange("p (h e) -> p h e", e=65)
                nc.vector.memset(
                    vv[:, :, 64:65].rearrange("p h one -> p (h one)"), 1.0)

            # DRAM tiles for collectives
            ytl = [dramp.tile([DW, 512], BF16, tag=f"ytl{j}", name=f"ytl{j}")
                   for j in range(NJ)]
            ytf = [dramp.tile([2 * DW, 512], BF16, tag=f"ytf{j}",
                              name=f"ytf{j}") for j in range(3)]
            ytf3 = [dramp.tile([256, 512], BF16, tag=f"ytf3_{hp}",
                               name=f"ytf3_{hp}") for hp in range(4)]

            # ---------------- Phase 1: QKV + rmsnorm + rope ----------------
            with (
                tc.tile_pool(name="p1w", bufs=1) as p1w,
                tc.tile_pool(name="p1x", bufs=2) as p1x,
                tc.tile_pool(name="p1", bufs=2) as p1,
                tc.tile_pool(name="p1ps", bufs=2, space=bass.MemorySpace.PSUM) as p1ps,
            ):
                wsb = {}
                for name, wd in (("q", wqT_d), ("k", wkT_d), ("v", wvT_d)):
                    w = p1w.tile([128, NCT * DW], BF16, tag=f"w{name}",
                                 name=f"w{name}")
                    # scalar-engine HWDGE ring: parallel to sync's, so the
                    # weight loads overlap the first xa quarter load
                    nc.scalar.dma_start(
                        w[:].rearrange("p (ct d) -> p ct d", d=DW),
                        wd.rearrange("(ct p) d -> p ct d", p=128),
                    )
                    wsb[name] = w

                NQ = 4           # t-tiles per x-load quarter
                for quarter in range(NT // NQ):
                    # xT quarter: [c(128p x 16ct), 512t] contiguous rows
                    xa = p1x.tile([128, NCT * NQ * 128], BF16, tag="xa")
                    xav = xa[:].rearrange("p (ct t) -> p ct t", t=NQ * 128)
                    xsrc = (xT_d.rearrange("(ct p) t -> p ct t", p=128)
                            [:, :, quarter * NQ * 128:(quarter + 1) * NQ * 128])
                    # split halves so the first matmuls start sooner
                    nc.sync.dma_start(xav[:, 0:8, :], xsrc[:, 0:8, :])
                    nc.sync.dma_start(xav[:, 8:16, :], xsrc[:, 8:16, :])
                    for tl in range(NQ):
                        tt = quarter * NQ + tl
                        cos_t = p1.tile([128, DW], F32, tag="cos")
                        sin_t = p1.tile([128, DW], F32, tag="sin")
                        nc.sync.dma_start(cos_t[:], cos_d[ts(tt, 128), :])
                        nc.sync.dma_start(sin_t[:], sin_d[ts(tt, 128), :])

                        ps = {}
                        for name in ("q", "k", "v"):
                            p = p1ps.tile([128, DW], F32, tag=f"ps{name}",
                                          name=f"ps{name}")
                            for ct in range(NCT):
                                nc.tensor.matmul(
                                    p[:],
                                    xav[:, ct, ts(tl, 128)],
                                    wsb[name][:, ts(ct, DW)],
                                    start=(ct == 0),
                                    stop=(ct == NCT - 1),
                                )
                            ps[name] = p

                        # V: straight to v65 (bf16), interleaved per head
                        nc.vector.tensor_copy(
                            v65[tt][:].rearrange("p (h e) -> p h e", e=65)
                            [:, :, 0:64],
                            ps["v"][:].rearrange("p (h d) -> p h d", d=64),
                        )

                        for name, dst in (("q", qtd), ("k", ktd)):
                            pq = ps[name]
                            pqv = pq[:].rearrange("p (h d) -> p h d", d=64)
                            sq = p1.tile([128, DW], F32, tag="sq", bufs=3)
                            nc.scalar.activation(sq[:], pq[:], AF.Square)
                            ssq = p1.tile([128, HPC], F32, tag="ssq", bufs=3)
                            nc.vector.tensor_reduce(
                                ssq[:],
                                sq[:].rearrange("p (h d) -> p h d", d=64),
                                axis=mybir.AxisListType.X,
                                op=mybir.AluOpType.add)
                            sig = p1.tile([128, HPC], F32, tag="sig", bufs=3)
                            nc.scalar.activation(
                                sig[:], ssq[:], AF.Sqrt,
                                bias=eps_sb[:], scale=1.0 / HEAD_DIM)
                            rfac = p1.tile([128, HPC], F32, tag="rfac", bufs=3)
                            nc.vector.reciprocal(rfac[:], sig[:])
                            rb = rfac[:].rearrange(
                                "p (h one) -> p h one", one=1
                            ).broadcast_to([128, HPC, 64])
                            qn = p1.tile([128, DW], F32, tag="qn", bufs=3)
                            qnv = qn[:].rearrange("p (h d) -> p h d", d=64)
                            nc.vector.tensor_mul(qnv, pqv, rb)
                            # rope: m1 = qn*cos; m2 = swap(qn)*sin(signed);
                            # m1b = bf16(m1 + m2)
                            m1 = p1.tile([128, DW], F32, tag="m1", bufs=3)
                            nc.vector.tensor_mul(m1[:], qn[:], cos_t[:])
                            m2 = p1.tile([128, DW], F32, tag="m2", bufs=3)
                            qh = qn[:].rearrange("p (h two d) -> p h two d",
                                                 two=2, d=32)
                            sh = sin_t[:].rearrange("p (h two d) -> p h two d",
                                                    two=2, d=32)
                            mh = m2[:].rearrange("p (h two d) -> p h two d",
                                                 two=2, d=32)
                            nc.vector.tensor_mul(mh[:, :, 0, :], qh[:, :, 1, :],
                                                 sh[:, :, 0, :])
                            nc.vector.tensor_mul(mh[:, :, 1, :], qh[:, :, 0, :],
                                                 sh[:, :, 1, :])
                            m1b = p1.tile([128, DW], BF16, tag="m1b", bufs=3)
                            nc.vector.tensor_add(m1b[:], m1[:], m2[:])
                            for db in range(4):
                                tp = p1ps.tile([128, 128], BF16, tag="tp",
                                               bufs=2)
                                nc.tensor.transpose(
                                    tp[:], m1b[:, ts(db, 128)], identb[:])
                                nc.vector.tensor_copy(
                                    dst[db][:, ts(tt, 128)], tp[:])

            # ---------------- Phase 2: attention + out-proj ----------------
            with (
                tc.tile_pool(name="p2w", bufs=1) as p2w,
                tc.tile_pool(name="p2", bufs=2) as p2,
                tc.tile_pool(name="p2sc", bufs=2, space=bass.MemorySpace.PSUM) as p2sc,
                tc.tile_pool(name="p2y", bufs=1, space=bass.MemorySpace.PSUM) as p2y,
                tc.tile_pool(name="p2bc", bufs=1, space=bass.MemorySpace.PSUM) as p2bc,
                tc.tile_pool(name="p2po", bufs=1, space=bass.MemorySpace.PSUM) as p2po,
            ):
                wosb = []
                for lt in range(8):
                    w = p2w.tile([128, CH], BF16, tag=f"wo{lt}", name=f"wo{lt}")
                    nc.sync.dma_start(w[:], woT_d[ts(lt, 128), :])
                    wosb.append(w)

                def attn(j, hp):
                    smax = (j + 1) * 4
                    pys = []
                    for e in range(2):
                        pys.append(p2y.tile([65, 512], F32, tag=f"py{e}",
                                            name=f"py{e}"))
                    for si in range(smax):
                        a = si - 4 * j
                        nlo = 128 * a if a > 0 else 0
                        sc = p2sc.tile([128, 1024], F32, tag="sc")
                        for e in range(2):
                            nc.tensor.matmul(
                                sc[:, e * 512 + nlo:(e + 1) * 512],
                                ktd[hp][64 * e:64 * e + 64, ts(si, 128)],
                                qtd[hp][64 * e:64 * e + 64,
                                        j * 512 + nlo:(j + 1) * 512],
                                start=True, stop=True,
                            )
                        pt = p2.tile([128, 1024], BF16, tag="pt", bufs=4)
                        if nlo:
                            scv = sc[:].rearrange("p (e c) -> p e c", e=2)
                            ptv = pt[:].rearrange("p (e c) -> p e c", e=2)
                            nc.scalar.activation(
                                ptv[:, :, nlo:512], scv[:, :, nlo:512],
                                AF.Exp, scale=1.0 / np.sqrt(HEAD_DIM))
                        else:
                            nc.scalar.activation(
                                pt[:], sc[:], AF.Exp,
                                scale=1.0 / np.sqrt(HEAD_DIM))
                        if a >= 0:
                            ptv = pt[:].rearrange("p (e c) -> p e c", e=2)
                            nc.vector.tensor_mul(
                                ptv[:, :, nlo:nlo + 128],
                                ptv[:, :, nlo:nlo + 128],
                                tri[:].rearrange("p (one c) -> p one c", one=1)
                                .broadcast_to([128, 2, 128]))
                        for e in range(2):
                            h = 2 * hp + e
                            nc.tensor.matmul(
                                pys[e][:, nlo:512],
                                v65[si][:, 65 * h:65 * h + 65],
                                pt[:, e * 512 + nlo:(e + 1) * 512],
                                start=(si == 0),
                                stop=(si == smax - 1),
                            )
                    for e in range(2):
                        ystage = p2.tile([65, 512], F32R, tag="ystage", bufs=4)
                        nc.vector.tensor_copy(ystage[:], pys[e][:])
                        bc = p2bc.tile([64, 512], F32, tag="bc")
                        nc.tensor.matmul(bc[:], onesr[64:65, :],
                                         ystage[64:65, :])
                        bcr = p2.tile([64, 512], F32, tag="bcr", bufs=4)
                        nc.vector.reciprocal(bcr[:], bc[:])
                        ynt = p2.tile([64, 512], BF16, tag="ynt", bufs=4)
                        nc.vector.tensor_mul(ynt[:], ystage[0:64, :], bcr[:])
                        nc.sync.dma_start(
                            ytl[j][128 * hp + 64 * e:128 * hp + 64 * e + 64, :],
                            ynt[:])

                def outproj(j, pieces):
                    for tt in range(4):
                        for cc in range(2):
                            po = p2po.tile([128, 512], F32, tag="po")
                            for i, (y, g) in enumerate(pieces):
                                nc.tensor.matmul(
                                    po[:],
                                    y[:, ts(tt, 128)],
                                    wosb[g][:, ts(cc, 512)],
                                    start=(i == 0),
                                    stop=(i == 7),
                                )
                            osb = p2.tile([128, 512], F32, tag="osb", bufs=3)
                            nc.vector.tensor_copy(osb[:], po[:])
                            nc.scalar.dma_start(
                                out_d[j * 512 + tt * 128:j * 512 + tt * 128 + 128,
                                      ts(cc, 512)], osb[:])

                def load_yts(j):
                    pieces = []
                    for lt in range(8):
                        y = p2.tile([128, 512], BF16, tag=f"yts{lt}",
                                    name=f"yts{lt}", bufs=2)
                        nc.sync.dma_start(y[:], ytf[j][ts(lt, 128), :])
                        pieces.append((y, lt))
                    return pieces

                # attention j-outer; per-j AG; out-proj trails by >=1 j so the
                # AG latency hides under the next j's attention
                # tile_wait_until pushes AG-dependent work later on the
                # scheduler's internal timeline — its collective cost estimate
                # is near zero, so without this the out-proj gets wedged into
                # the engine FIFOs ~40us early and head-of-line blocks them
                for j in range(4):
                    for hp in range(4):
                        attn(j, hp)
                        if j == 3:
                            nc.gpsimd.collective_compute(
                                "AllGather", mybir.AluOpType.bypass,
                                replica_groups=groups,
                                ins=[ytl[3][ts(hp, 128), :]],
                                outs=[ytf3[hp][:]],
                            )
                        if j == 3 and hp == 1:
                            with tc.tile_wait_until(0.33):
                                outproj(1, load_yts(1))
                        if j == 3 and hp == 3:
                            with tc.tile_wait_until(0.40):
                                outproj(2, load_yts(2))
                    if j < 3:
                        nc.gpsimd.collective_compute(
                            "AllGather", mybir.AluOpType.bypass,
                            replica_groups=groups,
                            ins=[ytl[j][:]], outs=[ytf[j][:]],
                        )
                    if j == 2:
                        with tc.tile_wait_until(0.27):
                            outproj(0, load_yts(0))
                with tc.tile_wait_until(0.42):
                    p3 = []
                    for hp in range(4):
                        for half in range(2):
                            y = p2.tile([128, 512], BF16,
                                        tag=f"y3_{hp}_{half}",
                                        name=f"y3_{hp}_{half}", bufs=1)
                            nc.sync.dma_start(y[:], ytf3[hp][ts(half, 128), :])
                            p3.append((y, 4 * half + hp))
                    outproj(3, p3)

    nc.compile()
    return nc


def host_tables(T_=T):
    inv_freq = 1.0 / (ROPE_BASE ** (np.arange(0, HEAD_DIM, 2, dtype=np.float32)
                                    / HEAD_DIM))
    t = np.arange(T_, dtype=np.float32)
    freqs = np.outer(t, inv_freq)
    cos = np.cos(freqs).astype(np.float32)
    sin = np.sin(freqs).astype(np.float32)
    cosf = np.tile(np.concatenate([cos, cos], axis=1), (1, HPC))
    sinf = np.tile(np.concatenate([sin, -sin], axis=1), (1, HPC))
    tri = (np.arange(128)[None, :] >= np.arange(128)[:, None])
    return (np.ascontiguousarray(cosf), np.ascontiguousarray(sinf),
            np.ascontiguousarray(tri))


def make_in_maps(x, w_qkv, w_out, T_=T, num_devices=N_CORES):
    import ml_dtypes
    bf16 = ml_dtypes.bfloat16
    x = np.asarray(x, dtype=np.float32)
    w_qkv = np.asarray(w_qkv, dtype=np.float32)
    w_out = np.asarray(w_out, dtype=np.float32)
    cosf, sinf, tri = host_tables(T_)
    in_maps = []
    for c in range(num_devices):
        b, hg = c // 2, c % 2
        sl = slice(hg * DW, (hg + 1) * DW)
        in_maps.append({
            "xT": np.ascontiguousarray(x[b].T).astype(bf16),
            "wqT": np.ascontiguousarray(w_qkv[0 * N_LATENT:, :][sl].T).astype(bf16),
            "wkT": np.ascontiguousarray(w_qkv[1 * N_LATENT:, :][sl].T).astype(bf16),
            "wvT": np.ascontiguousarray(w_qkv[2 * N_LATENT:, :][sl].T).astype(bf16),
            "woutT": np.ascontiguousarray(
                w_out[hg * C // 2:(hg + 1) * C // 2, :].T).astype(bf16),
            "cosf": cosf,
            "sinf": sinf,
            "tri": tri.astype(bf16),
        })
    return in_maps


_NC = None


def kernel(x, w_qkv, w_out):
    global _NC
    if _NC is None:
        _NC = build_nc()
    from concourse.bass_utils import run_bass_kernel_spmd
    in_maps = make_in_maps(x, w_qkv, w_out)
    res = run_bass_kernel_spmd(_NC, in_maps, list(range(N_CORES))).results
    B = 4
    out = np.empty((B, T, N_EMBD), dtype=np.float32)
    for c in range(N_CORES):
        b, hg = c // 2, c % 2
        out[b, :, hg * N_EMBD // 2:(hg + 1) * N_EMBD // 2] = res[c]["out_half"]
    return out
